# revision 5
# baseline (speedup 1.0000x reference)
"""PoseGNN Trainium2 kernel: 3-stage SPMD pipeline over 8 NeuronCores.

Sharding strategy (per spec sharding_hint): nodes are partitioned into
128-node tiles, tiles are bin-packed across the 8 cores by incident-edge
count, and each GraphConv aggregation's "halo exchange" of remote node
features is performed by the orchestrating host between device stages
(gather of message rows into a dst-sorted, degree-padded layout).

Stage L1: h1 = lrelu(x @ W1.T + b1); m1 = h1 @ Wrel.T; r1b = h1 @ Wroot.T + brel
Stage L2: agg1 = segment_sum(m1[src]); h2 = lrelu(agg1 + r1b); m2, r2b
Stage L3: agg2 = segment_sum(m2[src]); h3 = lrelu(agg2 + r2b); pos = h3 @ Wp.T + bp

Aggregation on device: each dst gets 32 fixed message slots (padded with a
zero row); the tail of higher-degree nodes goes through a small
one-hot-matmul overflow path. Main path is a log-tree DVE reduction.
"""
import sys
import time

sys.path.insert(0, "/opt/trn_rl_repo")

import numpy as np

import concourse.bass as bass
import concourse.bacc as bacc
import concourse.tile as tile
import concourse.mybir as mybir
from concourse.bass_utils import run_bass_kernel_spmd

F32 = mybir.dt.float32
P = 128
N_CORES = 8
N_NODES = 50000
F0, F1, F2, F3 = 2048, 128, 64, 32
TILES_PER_CORE = 49
NPC = TILES_PER_CORE * P          # 6272 nodes per core
NPAD = N_CORES * NPC              # 50176
N_TILES = NPAD // P               # 392
DEG = 32                          # main message slots per dst node
SLOPE = 0.01
ZROW = NPAD                       # index of the all-zero pad row in m*_full_ext

_cache = {}


# ---------------------------------------------------------------- host prep

def _build_partition(edge_index):
    src = np.asarray(edge_index[0], dtype=np.int64)
    dst = np.asarray(edge_index[1], dtype=np.int64)
    E = src.shape[0]

    # edge count per original 128-node tile (tiles 0..390 real, 391 empty pad)
    tile_of_dst = dst // P
    cnt = np.bincount(tile_of_dst, minlength=N_TILES)

    # greedy bin-pack tiles (by desc count) onto cores, 49 tiles each
    order = np.argsort(-cnt, kind="stable")
    core_tiles = [[] for _ in range(N_CORES)]
    core_load = np.zeros(N_CORES, dtype=np.int64)
    for t in order:
        cand = [c for c in range(N_CORES) if len(core_tiles[c]) < TILES_PER_CORE]
        c = min(cand, key=lambda cc: core_load[cc])
        core_tiles[c].append(t)
        core_load[c] += cnt[t]
    # slot s of every core has ~matching edge count (tiles were taken in desc
    # order, so each core's list is already desc)
    for c in range(N_CORES):
        core_tiles[c].sort(key=lambda t: -cnt[t])

    # node permutation: new id = c*NPC + s*128 + j  <->  old tile id*128 + j
    new2old = np.full(NPAD, -1, dtype=np.int64)
    for c in range(N_CORES):
        for s, t in enumerate(core_tiles[c]):
            base_new = c * NPC + s * P
            base_old = t * P
            n_valid = max(0, min(P, N_NODES - base_old))
            if n_valid > 0:
                new2old[base_new:base_new + n_valid] = np.arange(
                    base_old, base_old + n_valid)
    old2new = np.empty(N_NODES, dtype=np.int64)
    valid = new2old >= 0
    old2new[new2old[valid]] = np.nonzero(valid)[0]

    dst_new = old2new[dst]
    src_new = old2new[src]

    # sort edges by destination; rank within each destination group
    o = np.argsort(dst_new, kind="stable")
    dsts = dst_new[o]
    srcs = src_new[o]
    uniq, starts, counts = np.unique(dsts, return_index=True, return_counts=True)
    rank = np.arange(E) - np.repeat(starts, counts)

    # main path: first DEG edges of each dst -> fixed slots
    SRC_all = np.full((NPAD, DEG), ZROW, dtype=np.int32)
    main = rank < DEG
    SRC_all[dsts[main], rank[main]] = srcs[main]

    # overflow path: remaining edges, grouped by global tile (= dst_new//128)
    ov = ~main
    ov_dst = dsts[ov]
    ov_src = srcs[ov]
    ov_tile = ov_dst // P          # global tile id = c*49 + s
    ov_cnt = np.bincount(ov_tile, minlength=N_TILES)
    # shared per-slot chunk schedule: max over cores
    oc_sched = np.zeros(TILES_PER_CORE, dtype=np.int64)
    for s in range(TILES_PER_CORE):
        for c in range(N_CORES):
            g = c * TILES_PER_CORE + s
            oc_sched[s] = max(oc_sched[s], -(-ov_cnt[g] // P))
    oc_off = np.concatenate([[0], np.cumsum(oc_sched)])
    OC_TOTAL = int(oc_off[-1])

    ovf_src = np.full((N_CORES, P, max(OC_TOTAL, 1)), ZROW, dtype=np.int32)
    ovf_dstrel = np.full((N_CORES, P, max(OC_TOTAL, 1)), -1.0, dtype=np.float32)
    oo = np.argsort(ov_tile, kind="stable")
    ov_dst = ov_dst[oo]
    ov_src = ov_src[oo]
    ov_tile = ov_tile[oo]
    gstart = np.concatenate([[0], np.cumsum(np.bincount(ov_tile, minlength=N_TILES))])
    for g in range(N_TILES):
        a, b = gstart[g], gstart[g + 1]
        if a == b:
            continue
        c, s = divmod(g, TILES_PER_CORE)
        k = np.arange(b - a)
        ch = oc_off[s] + k // P
        pp = k % P
        ovf_src[c, pp, ch] = ov_src[a:b]
        ovf_dstrel[c, pp, ch] = (ov_dst[a:b] % P).astype(np.float32)

    # per-core SRC rows in device strip layout [P, 49, DEG]
    SRC_core = SRC_all.reshape(N_CORES, TILES_PER_CORE, P, DEG).transpose(0, 2, 1, 3)

    return dict(
        new2old=new2old, old2new=old2new,
        SRC_core=np.ascontiguousarray(SRC_core),
        ovf_src=ovf_src, ovf_dstrel=ovf_dstrel,
        oc_sched=[int(v) for v in oc_sched], oc_off=[int(v) for v in oc_off],
        OC_TOTAL=OC_TOTAL,
    )


# ---------------------------------------------------------------- programs

def _lrelu_emit(nc, pool, out_ap, v_ap, w, wmax=None):
    """out = max(v, SLOPE*v); v may be SBUF or PSUM."""
    t2 = pool.tile([P, wmax or w], F32, tag="lrelu_tmp")
    nc.scalar.activation(out=t2[:, :w], in_=v_ap,
                         func=mybir.ActivationFunctionType.Copy, scale=SLOPE)
    nc.vector.tensor_tensor(out=out_ap, in0=v_ap, in1=t2[:, :w],
                            op=mybir.AluOpType.max)


def _build_l1():
    nc = bacc.Bacc("TRN2", target_bir_lowering=False, debug=False,
                   num_devices=N_CORES)
    xt_in = nc.dram_tensor("xt", [F0, NPC], F32, kind="ExternalInput")
    w1t_in = nc.dram_tensor("w1t", [F0, F1], F32, kind="ExternalInput")
    b1_in = nc.dram_tensor("b1", [P, 1], F32, kind="ExternalInput")
    w1rel_in = nc.dram_tensor("w1rel", [F1, F2], F32, kind="ExternalInput")
    w1root_in = nc.dram_tensor("w1root", [F1, F2], F32, kind="ExternalInput")
    b1rel_in = nc.dram_tensor("b1rel", [P, F2], F32, kind="ExternalInput")
    m1_out = nc.dram_tensor("m1", [P, TILES_PER_CORE * F2], F32, kind="ExternalOutput")
    r1_out = nc.dram_tensor("r1b", [P, TILES_PER_CORE * F2], F32, kind="ExternalOutput")

    KC = F0 // P  # 16 contraction chunks
    widths = [512] * (NPC // 512) + ([NPC % 512] if NPC % 512 else [])

    with tile.TileContext(nc) as tc:
        with tc.tile_pool(name="wsb", bufs=1) as wsb, \
             tc.tile_pool(name="xp", bufs=2) as xp, \
             tc.tile_pool(name="hsb", bufs=1) as hsb, \
             tc.tile_pool(name="epi", bufs=2) as epi, \
             tc.tile_pool(name="psh", bufs=2, space="PSUM") as psh, \
             tc.tile_pool(name="psm", bufs=4, space="PSUM") as psm:

            w1t_sb = wsb.tile([P, KC * F1], F32)
            nc.sync.dma_start(
                w1t_sb[:].rearrange("p (k m) -> p k m", k=KC),
                w1t_in.ap().rearrange("(k p) m -> p k m", p=P))
            b1_sb = wsb.tile([P, 1], F32)
            nc.sync.dma_start(b1_sb[:], b1_in.ap())
            w1rel_sb = wsb.tile([P, F2], F32)
            nc.sync.dma_start(w1rel_sb[:], w1rel_in.ap())
            w1root_sb = wsb.tile([P, F2], F32)
            nc.sync.dma_start(w1root_sb[:], w1root_in.ap())
            b1rel_sb = wsb.tile([P, F2], F32)
            nc.sync.dma_start(b1rel_sb[:], b1rel_in.ap())

            h1_sb = hsb.tile([P, NPC], F32)
            m1_strip = hsb.tile([P, TILES_PER_CORE * F2], F32)
            r1_strip = hsb.tile([P, TILES_PER_CORE * F2], F32)

            xt_r = xt_in.ap().rearrange("(k p) n -> p k n", p=P)
            off = 0
            for w in widths:
                xt_t = xp.tile([P, KC * 512], F32, tag="xt")
                nc.sync.dma_start(
                    xt_t[:, :KC * w].rearrange("p (k n) -> p k n", k=KC),
                    xt_r[:, :, off:off + w])
                hp = psh.tile([P, 512], F32, tag="hp")
                for k in range(KC):
                    nc.tensor.matmul(
                        hp[:, :w],
                        lhsT=w1t_sb[:, k * F1:(k + 1) * F1],
                        rhs=xt_t[:, k * w:(k + 1) * w],
                        start=(k == 0), stop=(k == KC - 1))
                a = epi.tile([P, 512], F32, tag="a")
                nc.scalar.activation(out=a[:, :w], in_=hp[:, :w],
                                     func=mybir.ActivationFunctionType.Identity,
                                     bias=b1_sb[:, :1], scale=1.0)
                _lrelu_emit(nc, epi, h1_sb[:, off:off + w], a[:, :w], w, wmax=512)
                off += w

            for s in range(TILES_PER_CORE):
                h1_t = h1_sb[:, s * P:(s + 1) * P]
                m1p = psm.tile([P, F2], F32, tag="mm")
                nc.tensor.matmul(m1p[:], lhsT=h1_t, rhs=w1rel_sb[:],
                                 start=True, stop=True)
                nc.scalar.copy(m1_strip[:, s * F2:(s + 1) * F2], m1p[:])
                r1p = psm.tile([P, F2], F32, tag="mm")
                nc.tensor.matmul(r1p[:], lhsT=h1_t, rhs=w1root_sb[:],
                                 start=True, stop=True)
                nc.vector.tensor_tensor(out=r1_strip[:, s * F2:(s + 1) * F2],
                                        in0=r1p[:], in1=b1rel_sb[:],
                                        op=mybir.AluOpType.add)
            nc.sync.dma_start(m1_out.ap(), m1_strip[:])
            nc.sync.dma_start(r1_out.ap(), r1_strip[:])
    nc.compile()
    return nc


def _build_conv(fin, oc_sched, oc_off, oc_total, last):
    """Conv stage: messages [P, 49*DEG*fin] -> h = lrelu(sum + ovf + rb).
    If last: emit position [P, 49*3]; else emit m/rb at width fin//2."""
    fout = 3 if last else fin // 2
    nc = bacc.Bacc("TRN2", target_bir_lowering=False, debug=False,
                   num_devices=N_CORES)
    msg_in = nc.dram_tensor("msg", [P, TILES_PER_CORE * DEG * fin], F32,
                            kind="ExternalInput")
    rb_in = nc.dram_tensor("rb", [P, TILES_PER_CORE * fin], F32,
                           kind="ExternalInput")
    ovm_in = nc.dram_tensor("ovm", [P, max(oc_total, 1) * fin], F32,
                            kind="ExternalInput")
    ovd_in = nc.dram_tensor("ovd", [P, max(oc_total, 1)], F32,
                            kind="ExternalInput")
    iota_in = nc.dram_tensor("iota", [P, P], F32, kind="ExternalInput")
    ident_in = nc.dram_tensor("ident", [P, P], F32, kind="ExternalInput")
    wa_in = nc.dram_tensor("wa", [fin, fout], F32, kind="ExternalInput")
    if not last:
        wb_in = nc.dram_tensor("wb", [fin, fout], F32, kind="ExternalInput")
    bia_in = nc.dram_tensor("bia", [P, fout], F32, kind="ExternalInput")
    if last:
        out_t = nc.dram_tensor("pos", [P, TILES_PER_CORE * 3], F32,
                               kind="ExternalOutput")
    else:
        m_out = nc.dram_tensor("m", [P, TILES_PER_CORE * fout], F32,
                               kind="ExternalOutput")
        rb_out = nc.dram_tensor("rbo", [P, TILES_PER_CORE * fout], F32,
                                kind="ExternalOutput")

    G = 4                       # dst tiles per msg DMA
    TW = DEG * fin              # msg columns per dst tile

    with tile.TileContext(nc) as tc:
        with tc.tile_pool(name="wsb", bufs=1) as wsb, \
             tc.tile_pool(name="msgp", bufs=2) as msgp, \
             tc.tile_pool(name="ovp", bufs=2) as ovp, \
             tc.tile_pool(name="ohp", bufs=2) as ohp, \
             tc.tile_pool(name="epi", bufs=2) as epi, \
             tc.tile_pool(name="hsb", bufs=1) as hsb, \
             tc.tile_pool(name="pso", bufs=2, space="PSUM") as pso, \
             tc.tile_pool(name="pst", bufs=2, space="PSUM") as pst, \
             tc.tile_pool(name="psm", bufs=4, space="PSUM") as psm:

            iota_sb = wsb.tile([P, P], F32)
            nc.sync.dma_start(iota_sb[:], iota_in.ap())
            ident_sb = wsb.tile([P, P], F32)
            nc.sync.dma_start(ident_sb[:], ident_in.ap())
            rb_sb = wsb.tile([P, TILES_PER_CORE * fin], F32)
            nc.sync.dma_start(rb_sb[:], rb_in.ap())
            ovd_sb = wsb.tile([P, max(oc_total, 1)], F32)
            nc.sync.dma_start(ovd_sb[:], ovd_in.ap())
            wa_sb = wsb.tile([fin, fout], F32)
            nc.sync.dma_start(wa_sb[:], wa_in.ap())
            if not last:
                wb_sb = wsb.tile([fin, fout], F32)
                nc.sync.dma_start(wb_sb[:], wb_in.ap())
            bia_sb = wsb.tile([P, fout], F32)
            nc.sync.dma_start(bia_sb[:], bia_in.ap())

            h_sb = hsb.tile([P, TILES_PER_CORE * fin], F32)
            out_strip = hsb.tile([P, TILES_PER_CORE * fout], F32)
            if not last:
                rbo_strip = hsb.tile([P, TILES_PER_CORE * fout], F32)

            msg_t = None
            for s in range(TILES_PER_CORE):
                if s % G == 0:
                    gw = min(G, TILES_PER_CORE - s) * TW
                    msg_t = msgp.tile([P, G * TW], F32, tag="msg")
                    nc.sync.dma_start(msg_t[:, :gw],
                                      msg_in.ap()[:, s * TW:s * TW + gw])
                base = (s % G) * TW
                # log-tree reduction over the DEG axis (layout [j, fin])
                h = DEG // 2
                while h >= 1:
                    nc.vector.tensor_tensor(
                        out=msg_t[:, base:base + h * fin],
                        in0=msg_t[:, base:base + h * fin],
                        in1=msg_t[:, base + h * fin:base + 2 * h * fin],
                        op=mybir.AluOpType.add)
                    h //= 2
                s_ap = msg_t[:, base:base + fin]

                oc = oc_sched[s]
                v = epi.tile([P, fin], F32, tag="v")
                nc.vector.tensor_tensor(
                    out=v[:], in0=s_ap,
                    in1=rb_sb[:, s * fin:(s + 1) * fin],
                    op=mybir.AluOpType.add)
                if oc > 0:
                    f0 = oc_off[s]
                    ot = ovp.tile([P, max(oc_sched) * fin], F32, tag="ovf")
                    nc.sync.dma_start(ot[:, :oc * fin],
                                      ovm_in.ap()[:, f0 * fin:(f0 + oc) * fin])
                    oh = ohp.tile([P, max(oc_sched) * P], F32, tag="oh")
                    nc.vector.tensor_tensor(
                        out=oh[:, :oc * P].rearrange("p (c d) -> p c d", c=oc),
                        in0=ovd_sb[:, f0:f0 + oc].to_broadcast([P, oc, P]),
                        in1=iota_sb[:, None, :].to_broadcast([P, oc, P]),
                        op=mybir.AluOpType.is_equal)
                    op_ps = pso.tile([P, fin], F32, tag="ops")
                    for c2 in range(oc):
                        nc.tensor.matmul(
                            op_ps[:],
                            lhsT=oh[:, c2 * P:(c2 + 1) * P],
                            rhs=ot[:, c2 * fin:(c2 + 1) * fin],
                            start=(c2 == 0), stop=(c2 == oc - 1))
                    nc.vector.tensor_tensor(out=v[:], in0=v[:], in1=op_ps[:],
                                            op=mybir.AluOpType.add)
                _lrelu_emit(nc, epi, h_sb[:, s * fin:(s + 1) * fin], v[:], fin)

            for s in range(TILES_PER_CORE):
                trp = pst.tile([fin, P], F32, tag="tr")
                nc.tensor.transpose(trp[:], h_sb[:, s * fin:(s + 1) * fin],
                                    ident_sb[:])
                ht = epi.tile([fin, P], F32, tag="ht")
                nc.scalar.copy(ht[:], trp[:])
                mp = psm.tile([P, fout], F32, tag="mm")
                nc.tensor.matmul(mp[:], lhsT=ht[:], rhs=wa_sb[:],
                                 start=True, stop=True)
                if last:
                    nc.vector.tensor_tensor(
                        out=out_strip[:, s * fout:(s + 1) * fout],
                        in0=mp[:], in1=bia_sb[:], op=mybir.AluOpType.add)
                else:
                    nc.scalar.copy(out_strip[:, s * fout:(s + 1) * fout], mp[:])
                    rp = psm.tile([P, fout], F32, tag="mm")
                    nc.tensor.matmul(rp[:], lhsT=ht[:], rhs=wb_sb[:],
                                     start=True, stop=True)
                    nc.vector.tensor_tensor(
                        out=rbo_strip[:, s * fout:(s + 1) * fout],
                        in0=rp[:], in1=bia_sb[:], op=mybir.AluOpType.add)

            if last:
                nc.sync.dma_start(out_t.ap(), out_strip[:])
            else:
                nc.sync.dma_start(m_out.ap(), out_strip[:])
                nc.sync.dma_start(rb_out.ap(), rbo_strip[:])
    nc.compile()
    return nc


# ---------------------------------------------------------------- runner

class _Results:
    def __init__(self, results):
        self.results = results


class _Runner:
    """Persistent PJRT runner for one compiled Bass program on 8 cores.

    Mirrors bass2jax.run_bass_via_pjrt's multi-core branch but keeps the
    jitted shard_map callable alive across calls (no per-call retrace)."""

    def __init__(self, nc):
        import jax
        from concourse import bass2jax
        from jax.experimental.shard_map import shard_map
        from jax.sharding import Mesh, PartitionSpec

        bass2jax.install_neuronx_cc_hook()
        self.nc = nc
        self.jax = jax
        partition_name = (nc.partition_id_tensor.name
                          if nc.partition_id_tensor else None)
        in_names, out_names, out_avals, zero_info = [], [], [], []
        for alloc in nc.m.functions[0].allocations:
            if not isinstance(alloc, mybir.MemoryLocationSet):
                continue
            name = alloc.memorylocations[0].name
            if alloc.kind == "ExternalInput":
                if name != partition_name:
                    in_names.append(name)
            elif alloc.kind == "ExternalOutput":
                shape = tuple(alloc.tensor_shape)
                dtype = mybir.dt.np(alloc.dtype)
                out_names.append(name)
                out_avals.append(jax.core.ShapedArray(shape, dtype))
                zero_info.append((shape, dtype))
        self.param_names = list(in_names)
        self.out_names = out_names
        self.out_shapes = [z[0] for z in zero_info]
        self.zero_info = zero_info
        n_params, n_outs = len(in_names), len(out_names)
        all_in = in_names + out_names
        if partition_name is not None:
            all_in.append(partition_name)

        def _body(*args):
            operands = list(args)
            if partition_name is not None:
                operands.append(bass2jax.partition_id_tensor())
            outs = bass2jax._bass_exec_p.bind(
                *operands,
                out_avals=tuple(out_avals),
                in_names=tuple(all_in),
                out_names=tuple(out_names),
                lowering_input_output_aliases=(),
                sim_require_finite=True,
                sim_require_nnan=True,
                nc=nc,
            )
            return tuple(outs)

        devices = jax.devices()[:N_CORES]
        self.mesh = Mesh(np.asarray(devices), ("core",))
        self.pspec = PartitionSpec("core")
        in_specs = (self.pspec,) * (n_params + n_outs)
        out_specs = (self.pspec,) * n_outs
        self.fn = jax.jit(
            shard_map(_body, mesh=self.mesh, in_specs=in_specs,
                      out_specs=out_specs, check_rep=False),
            donate_argnums=tuple(range(n_params, n_params + n_outs)),
            keep_unused=True)

    def concat_inputs(self, in_maps):
        return [np.concatenate([np.asarray(in_maps[c][nm])
                                for c in range(N_CORES)], axis=0)
                for nm in self.param_names]

    def zeros(self):
        return [np.zeros((N_CORES * s[0], *s[1:]), d) for s, d in self.zero_info]

    def __call__(self, in_maps):
        outs = self.fn(*self.concat_inputs(in_maps), *self.zeros())
        return _Results([
            {nm: np.asarray(outs[i]).reshape(N_CORES, *self.out_shapes[i])[c]
             for i, nm in enumerate(self.out_names)}
            for c in range(N_CORES)])


def _run(nc, in_maps, tries=3):
    key = id(nc)
    if key not in _cache:
        _cache[key] = _Runner(nc)
    for i in range(tries):
        try:
            return _cache[key](in_maps)
        except Exception:
            if i == tries - 1:
                raise
            time.sleep(10)


def _msgs_from(m_full_ext, SRC_core, fin):
    """Per-core message strips [P, 49*DEG*fin] from the global feature table."""
    out = []
    for c in range(N_CORES):
        g = m_full_ext[SRC_core[c]]          # [P, 49, DEG, fin]
        out.append(np.ascontiguousarray(g.reshape(P, -1)))
    return out


def _ovf_msgs_from(m_full_ext, ovf_src, fin):
    out = []
    for c in range(N_CORES):
        g = m_full_ext[ovf_src[c]]           # [P, OC_TOTAL, fin]
        out.append(np.ascontiguousarray(g.reshape(P, -1)))
    return out


def _strip_to_rows(strip, fin):
    """[P, 49*fin] device strip -> [NPC, fin] rows in new-id order."""
    return strip.reshape(P, TILES_PER_CORE, fin).transpose(1, 0, 2).reshape(NPC, fin)


def kernel(x, edge_index, lin1_w, lin1_b, conv1_w_rel, conv1_b_rel,
           conv1_w_root, conv4_w_rel, conv4_b_rel, conv4_w_root,
           pos_w, pos_b):
    if "part" not in _cache:
        _cache["part"] = _build_partition(edge_index)
    part = _cache["part"]

    if "l1" not in _cache:
        _cache["l1"] = _build_l1()
        _cache["l2"] = _build_conv(F2, part["oc_sched"], part["oc_off"],
                                   part["OC_TOTAL"], last=False)
        _cache["l3"] = _build_conv(F3, part["oc_sched"], part["oc_off"],
                                   part["OC_TOTAL"], last=True)

    new2old = part["new2old"]
    x = np.asarray(x, dtype=np.float32)

    # ---- stage L1 inputs
    w1t = np.ascontiguousarray(np.asarray(lin1_w, np.float32).T)      # [2048,128]
    b1c = np.ascontiguousarray(np.asarray(lin1_b, np.float32).reshape(P, 1))
    w1rel = np.ascontiguousarray(np.asarray(conv1_w_rel, np.float32).T)
    w1root = np.ascontiguousarray(np.asarray(conv1_w_root, np.float32).T)
    b1rel = np.ascontiguousarray(
        np.broadcast_to(np.asarray(conv1_b_rel, np.float32), (P, F2)))

    in_maps1 = []
    for c in range(N_CORES):
        ids = new2old[c * NPC:(c + 1) * NPC]
        xc = np.zeros((NPC, F0), np.float32)
        m = ids >= 0
        xc[m] = x[ids[m]]
        in_maps1.append({
            "xt": np.ascontiguousarray(xc.T),
            "w1t": w1t, "b1": b1c, "w1rel": w1rel, "w1root": w1root,
            "b1rel": b1rel,
        })
    res1 = _run(_cache["l1"], in_maps1)

    m1_full_ext = np.zeros((NPAD + 1, F2), np.float32)
    for c in range(N_CORES):
        m1_full_ext[c * NPC:(c + 1) * NPC] = _strip_to_rows(
            res1.results[c]["m1"], F2)

    # ---- stage L2
    iota = np.ascontiguousarray(
        np.broadcast_to(np.arange(P, dtype=np.float32), (P, P)))
    ident = np.eye(P, dtype=np.float32)
    w4rel = np.ascontiguousarray(np.asarray(conv4_w_rel, np.float32).T)
    w4root = np.ascontiguousarray(np.asarray(conv4_w_root, np.float32).T)
    b4rel = np.ascontiguousarray(
        np.broadcast_to(np.asarray(conv4_b_rel, np.float32), (P, F3)))

    msgs1 = _msgs_from(m1_full_ext, part["SRC_core"], F2)
    ovfs1 = _ovf_msgs_from(m1_full_ext, part["ovf_src"], F2)
    in_maps2 = []
    for c in range(N_CORES):
        in_maps2.append({
            "msg": msgs1[c],
            "rb": res1.results[c]["r1b"],
            "ovm": ovfs1[c],
            "ovd": np.ascontiguousarray(part["ovf_dstrel"][c]),
            "iota": iota, "ident": ident,
            "wa": w4rel, "wb": w4root, "bia": b4rel,
        })
    res2 = _run(_cache["l2"], in_maps2)

    m2_full_ext = np.zeros((NPAD + 1, F3), np.float32)
    for c in range(N_CORES):
        m2_full_ext[c * NPC:(c + 1) * NPC] = _strip_to_rows(
            res2.results[c]["m"], F3)

    # ---- stage L3
    posw = np.ascontiguousarray(np.asarray(pos_w, np.float32).T)      # [32, 3]
    posb = np.ascontiguousarray(
        np.broadcast_to(np.asarray(pos_b, np.float32), (P, 3)))
    msgs2 = _msgs_from(m2_full_ext, part["SRC_core"], F3)
    ovfs2 = _ovf_msgs_from(m2_full_ext, part["ovf_src"], F3)
    in_maps3 = []
    for c in range(N_CORES):
        in_maps3.append({
            "msg": msgs2[c],
            "rb": res2.results[c]["rbo"],
            "ovm": ovfs2[c],
            "ovd": np.ascontiguousarray(part["ovf_dstrel"][c]),
            "iota": iota, "ident": ident,
            "wa": posw, "bia": posb,
        })
    res3 = _run(_cache["l3"], in_maps3)

    pos_full = np.empty((NPAD, 3), np.float32)
    for c in range(N_CORES):
        pos_full[c * NPC:(c + 1) * NPC] = _strip_to_rows(
            res3.results[c]["pos"], 3)

    out = np.empty((N_NODES, 3), np.float32)
    valid = new2old >= 0
    out[new2old[valid]] = pos_full[valid]
    return out


# revision 15
# speedup vs baseline: 70854.2092x; 70854.2092x over previous
"""PoseGNN Trainium2 kernel: 3-stage SPMD pipeline over 8 NeuronCores.

Sharding strategy (per spec sharding_hint): nodes are partitioned into
128-node tiles, tiles are bin-packed across the 8 cores by incident-edge
count, and each GraphConv aggregation's "halo exchange" of remote node
features is performed by the orchestrating host between device stages
(gather of message rows into a dst-sorted, degree-padded layout).

Stage L1: h1 = lrelu(x @ W1.T + b1); m1 = h1 @ Wrel.T; r1b = h1 @ Wroot.T + brel
Stage L2: agg1 = segment_sum(m1[src]); h2 = lrelu(agg1 + r1b); m2, r2b
Stage L3: agg2 = segment_sum(m2[src]); h3 = lrelu(agg2 + r2b); pos = h3 @ Wp.T + bp

Aggregation on device: each dst gets 32 fixed message slots (padded with a
zero row); the tail of higher-degree nodes goes through a small
one-hot-matmul overflow path. Main path is a log-tree DVE reduction.
"""
import sys
import time

sys.path.insert(0, "/opt/trn_rl_repo")

import numpy as np

import concourse.bass as bass
import concourse.bacc as bacc
import concourse.tile as tile
import concourse.mybir as mybir
from concourse.bass_utils import run_bass_kernel_spmd

F32 = mybir.dt.float32
F32R = mybir.dt.float32r
P = 128
N_CORES = 8
N_NODES = 50000
F0, F1, F2, F3 = 2048, 128, 64, 32
TILES_PER_CORE = 49
NPC = TILES_PER_CORE * P          # 6272 nodes per core
NPAD = N_CORES * NPC              # 50176
N_TILES = NPAD // P               # 392
DEG = 32                          # main message slots per dst node
SLOPE = 0.01
ZROW = NPAD                       # index of the all-zero pad row in m*_full_ext

_cache = {}

# tuned via TimelineSim sweep
OPTS = dict(merge=True, ht_dve=True, ovf_act=False, G=2, msg_bufs=3,
            reduce_mode="split_fj")


# ---------------------------------------------------------------- host prep

def _build_partition(edge_index):
    src = np.asarray(edge_index[0], dtype=np.int64)
    dst = np.asarray(edge_index[1], dtype=np.int64)
    E = src.shape[0]

    # edge count per original 128-node tile (tiles 0..390 real, 391 empty pad)
    tile_of_dst = dst // P
    cnt = np.bincount(tile_of_dst, minlength=N_TILES)

    # greedy bin-pack tiles (by desc count) onto cores, 49 tiles each
    order = np.argsort(-cnt, kind="stable")
    core_tiles = [[] for _ in range(N_CORES)]
    core_load = np.zeros(N_CORES, dtype=np.int64)
    for t in order:
        cand = [c for c in range(N_CORES) if len(core_tiles[c]) < TILES_PER_CORE]
        c = min(cand, key=lambda cc: core_load[cc])
        core_tiles[c].append(t)
        core_load[c] += cnt[t]
    # slot s of every core has ~matching edge count (tiles were taken in desc
    # order, so each core's list is already desc)
    for c in range(N_CORES):
        core_tiles[c].sort(key=lambda t: -cnt[t])

    # node permutation: new id = c*NPC + s*128 + j  <->  old tile id*128 + j
    new2old = np.full(NPAD, -1, dtype=np.int64)
    for c in range(N_CORES):
        for s, t in enumerate(core_tiles[c]):
            base_new = c * NPC + s * P
            base_old = t * P
            n_valid = max(0, min(P, N_NODES - base_old))
            if n_valid > 0:
                new2old[base_new:base_new + n_valid] = np.arange(
                    base_old, base_old + n_valid)
    old2new = np.empty(N_NODES, dtype=np.int64)
    valid = new2old >= 0
    old2new[new2old[valid]] = np.nonzero(valid)[0]

    dst_new = old2new[dst]
    src_new = old2new[src]

    # sort edges by destination; rank within each destination group
    o = np.argsort(dst_new, kind="stable")
    dsts = dst_new[o]
    srcs = src_new[o]
    uniq, starts, counts = np.unique(dsts, return_index=True, return_counts=True)
    rank = np.arange(E) - np.repeat(starts, counts)

    # main path: first DEG edges of each dst -> fixed slots
    SRC_all = np.full((NPAD, DEG), ZROW, dtype=np.int32)
    main = rank < DEG
    SRC_all[dsts[main], rank[main]] = srcs[main]

    # overflow path: remaining edges, grouped by global tile (= dst_new//128)
    ov = ~main
    ov_dst = dsts[ov]
    ov_src = srcs[ov]
    ov_tile = ov_dst // P          # global tile id = c*49 + s
    ov_cnt = np.bincount(ov_tile, minlength=N_TILES)
    # shared per-slot chunk schedule: max over cores
    oc_sched = np.zeros(TILES_PER_CORE, dtype=np.int64)
    for s in range(TILES_PER_CORE):
        for c in range(N_CORES):
            g = c * TILES_PER_CORE + s
            oc_sched[s] = max(oc_sched[s], -(-ov_cnt[g] // P))
    oc_off = np.concatenate([[0], np.cumsum(oc_sched)])
    OC_TOTAL = int(oc_off[-1])

    ovf_src = np.full((N_CORES, P, max(OC_TOTAL, 1)), ZROW, dtype=np.int32)
    ovf_dstrel = np.full((N_CORES, P, max(OC_TOTAL, 1)), -1.0, dtype=np.float32)
    oo = np.argsort(ov_tile, kind="stable")
    ov_dst = ov_dst[oo]
    ov_src = ov_src[oo]
    ov_tile = ov_tile[oo]
    gstart = np.concatenate([[0], np.cumsum(np.bincount(ov_tile, minlength=N_TILES))])
    for g in range(N_TILES):
        a, b = gstart[g], gstart[g + 1]
        if a == b:
            continue
        c, s = divmod(g, TILES_PER_CORE)
        k = np.arange(b - a)
        ch = oc_off[s] + k // P
        pp = k % P
        ovf_src[c, pp, ch] = ov_src[a:b]
        ovf_dstrel[c, pp, ch] = (ov_dst[a:b] % P).astype(np.float32)

    # per-core SRC rows in device strip layout [P, 49, DEG]
    SRC_core = SRC_all.reshape(N_CORES, TILES_PER_CORE, P, DEG).transpose(0, 2, 1, 3)

    return dict(
        new2old=new2old, old2new=old2new,
        SRC_core=np.ascontiguousarray(SRC_core),
        ovf_src=ovf_src, ovf_dstrel=ovf_dstrel,
        oc_sched=[int(v) for v in oc_sched], oc_off=[int(v) for v in oc_off],
        OC_TOTAL=OC_TOTAL,
    )


# ---------------------------------------------------------------- programs

def _lrelu_emit(nc, pool, out_ap, v_ap, w, wmax=None):
    """out = max(v, SLOPE*v); v may be SBUF or PSUM."""
    t2 = pool.tile([P, wmax or w], F32, tag="lrelu_tmp")
    nc.scalar.activation(out=t2[:, :w], in_=v_ap,
                         func=mybir.ActivationFunctionType.Copy, scale=SLOPE)
    nc.vector.tensor_tensor(out=out_ap, in0=v_ap, in1=t2[:, :w],
                            op=mybir.AluOpType.max)


def _build_l1():
    nc = bacc.Bacc("TRN2", target_bir_lowering=False, debug=False,
                   num_devices=N_CORES)
    xt_in = nc.dram_tensor("xt", [F0, NPC], F32R, kind="ExternalInput")
    w1t_in = nc.dram_tensor("w1t", [F0, F1], F32R, kind="ExternalInput")
    b1_in = nc.dram_tensor("b1", [P, 1], F32, kind="ExternalInput")
    w1rel_in = nc.dram_tensor("w1rel", [F1, F2], F32, kind="ExternalInput")
    w1root_in = nc.dram_tensor("w1root", [F1, F2], F32, kind="ExternalInput")
    b1rel_in = nc.dram_tensor("b1rel", [P, F2], F32, kind="ExternalInput")
    m1_out = nc.dram_tensor("m1", [P, TILES_PER_CORE * F2], F32, kind="ExternalOutput")
    r1_out = nc.dram_tensor("r1b", [P, TILES_PER_CORE * F2], F32, kind="ExternalOutput")

    KC = F0 // P  # 16 contraction chunks
    widths = [512] * (NPC // 512) + ([NPC % 512] if NPC % 512 else [])

    with tile.TileContext(nc) as tc:
        with tc.tile_pool(name="wsb", bufs=1) as wsb, \
             tc.tile_pool(name="xp", bufs=2) as xp, \
             tc.tile_pool(name="hsb", bufs=1) as hsb, \
             tc.tile_pool(name="epi", bufs=2) as epi, \
             tc.tile_pool(name="psh", bufs=2, space="PSUM") as psh, \
             tc.tile_pool(name="psm", bufs=4, space="PSUM") as psm:

            w1t_sb = wsb.tile([P, KC * F1], F32R)
            nc.scalar.dma_start(
                w1t_sb[:].rearrange("p (k m) -> p k m", k=KC),
                w1t_in.ap().rearrange("(k p) m -> p k m", p=P))
            b1_sb = wsb.tile([P, 1], F32)
            nc.scalar.dma_start(b1_sb[:], b1_in.ap())
            w1rel_sb = wsb.tile([P, F2], F32)
            nc.scalar.dma_start(w1rel_sb[:], w1rel_in.ap())
            w1root_sb = wsb.tile([P, F2], F32)
            nc.scalar.dma_start(w1root_sb[:], w1root_in.ap())
            b1rel_sb = wsb.tile([P, F2], F32)
            nc.scalar.dma_start(b1rel_sb[:], b1rel_in.ap())

            h1_sb = hsb.tile([P, NPC], F32)
            m1_strip = hsb.tile([P, TILES_PER_CORE * F2], F32)
            r1_strip = hsb.tile([P, TILES_PER_CORE * F2], F32)

            xt_r = xt_in.ap().rearrange("(k p) n -> p k n", p=P)
            off = 0
            for nb, w in enumerate(widths):
                xt_t = xp.tile([P, KC * 512], F32R, tag="xt")
                nc.sync.dma_start(
                    xt_t[:, :KC * w].rearrange("p (k n) -> p k n", k=KC),
                    xt_r[:, :, off:off + w])
                hp = psh.tile([P, 512], F32, tag="hp")
                for k in range(KC):
                    nc.tensor.matmul(
                        hp[:, :w],
                        lhsT=w1t_sb[:, k * F1:(k + 1) * F1],
                        rhs=xt_t[:, k * w:(k + 1) * w],
                        start=(k == 0), stop=(k == KC - 1))
                a = epi.tile([P, 512], F32, tag="a")
                nc.scalar.activation(out=a[:, :w], in_=hp[:, :w],
                                     func=mybir.ActivationFunctionType.Identity,
                                     bias=b1_sb[:, :1], scale=1.0)
                _lrelu_emit(nc, epi, h1_sb[:, off:off + w], a[:, :w], w, wmax=512)
                # m1/r1 for the slots this block completed (overlap with stream)
                for s in range(off // P, (off + w) // P):
                    h1_t = h1_sb[:, s * P:(s + 1) * P]
                    m1p = psm.tile([P, F2], F32, tag="mm")
                    nc.tensor.matmul(m1p[:], lhsT=h1_t, rhs=w1rel_sb[:],
                                     start=True, stop=True)
                    nc.scalar.copy(m1_strip[:, s * F2:(s + 1) * F2], m1p[:])
                    r1p = psm.tile([P, F2], F32, tag="mm")
                    nc.tensor.matmul(r1p[:], lhsT=h1_t, rhs=w1root_sb[:],
                                     start=True, stop=True)
                    nc.vector.tensor_tensor(
                        out=r1_strip[:, s * F2:(s + 1) * F2],
                        in0=r1p[:], in1=b1rel_sb[:], op=mybir.AluOpType.add)
                off += w
            nc.sync.dma_start(m1_out.ap(), m1_strip[:])
            nc.sync.dma_start(r1_out.ap(), r1_strip[:])
    nc.compile()
    return nc


def _build_conv(fin, oc_sched, oc_off, oc_total, last, opts=None):
    """Conv stage: messages [P, 49*DEG*fin] -> h = lrelu(sum + ovf + rb).
    If last: emit position [P, 49*3]; else emit m/rb at width fin//2."""
    opts = opts or {}
    merge = opts.get("merge", True)
    ht_dve = opts.get("ht_dve", True)
    ovf_act = opts.get("ovf_act", True)
    G = opts.get("G", 4)
    msg_bufs = opts.get("msg_bufs", 2)
    fout = 3 if last else fin // 2
    nc = bacc.Bacc("TRN2", target_bir_lowering=False, debug=False,
                   num_devices=N_CORES)
    msg_in = nc.dram_tensor("msg", [P, TILES_PER_CORE * DEG * fin], F32,
                            kind="ExternalInput")
    rb_in = nc.dram_tensor("rb", [P, TILES_PER_CORE * fin], F32,
                           kind="ExternalInput")
    ovm_in = nc.dram_tensor("ovm", [P, max(oc_total, 1) * fin], F32,
                            kind="ExternalInput")
    ovd_in = nc.dram_tensor("ovd", [P, max(oc_total, 1)], F32,
                            kind="ExternalInput")
    iota_in = nc.dram_tensor("iota", [P, P], F32, kind="ExternalInput")
    ident_in = nc.dram_tensor("ident", [P, P], F32, kind="ExternalInput")
    wa_in = nc.dram_tensor("wa", [fin, fout], F32, kind="ExternalInput")
    if not last:
        wb_in = nc.dram_tensor("wb", [fin, fout], F32, kind="ExternalInput")
    bia_in = nc.dram_tensor("bia", [P, fout], F32, kind="ExternalInput")
    if last:
        out_t = nc.dram_tensor("pos", [P, TILES_PER_CORE * 3], F32,
                               kind="ExternalOutput")
    else:
        m_out = nc.dram_tensor("m", [P, TILES_PER_CORE * fout], F32,
                               kind="ExternalOutput")
        rb_out = nc.dram_tensor("rbo", [P, TILES_PER_CORE * fout], F32,
                                kind="ExternalOutput")

    TW = DEG * fin              # msg columns per dst tile

    with tile.TileContext(nc) as tc:
        with tc.tile_pool(name="wsb", bufs=1) as wsb, \
             tc.tile_pool(name="msgp", bufs=msg_bufs) as msgp, \
             tc.tile_pool(name="ovp", bufs=2) as ovp, \
             tc.tile_pool(name="ohp", bufs=2) as ohp, \
             tc.tile_pool(name="epi", bufs=2) as epi, \
             tc.tile_pool(name="hsb", bufs=1) as hsb, \
             tc.tile_pool(name="pso", bufs=2, space="PSUM") as pso, \
             tc.tile_pool(name="pst", bufs=2, space="PSUM") as pst, \
             tc.tile_pool(name="psm", bufs=4, space="PSUM") as psm:

            iota_sb = wsb.tile([P, P], F32)
            nc.scalar.dma_start(iota_sb[:], iota_in.ap())
            ident_sb = wsb.tile([P, P], F32)
            nc.scalar.dma_start(ident_sb[:], ident_in.ap())
            rb_sb = wsb.tile([P, TILES_PER_CORE * fin], F32)
            nc.scalar.dma_start(rb_sb[:], rb_in.ap())
            ovd_sb = wsb.tile([P, max(oc_total, 1)], F32)
            nc.scalar.dma_start(ovd_sb[:], ovd_in.ap())
            wa_sb = wsb.tile([fin, fout], F32)
            nc.scalar.dma_start(wa_sb[:], wa_in.ap())
            if not last:
                wb_sb = wsb.tile([fin, fout], F32)
                nc.scalar.dma_start(wb_sb[:], wb_in.ap())
            bia_sb = wsb.tile([P, fout], F32)
            nc.scalar.dma_start(bia_sb[:], bia_in.ap())

            h_sb = hsb.tile([P, TILES_PER_CORE * fin], F32)
            out_strip = hsb.tile([P, TILES_PER_CORE * fout], F32)
            if not last:
                rbo_strip = hsb.tile([P, TILES_PER_CORE * fout], F32)

            def emit_tail(s):
                trp = pst.tile([fin, P], F32, tag="tr", name="trp")
                nc.tensor.transpose(trp[:], h_sb[:, s * fin:(s + 1) * fin],
                                    ident_sb[:])
                ht = epi.tile([fin, P], F32, tag="ht", name="ht")
                if ht_dve:
                    nc.vector.tensor_copy(ht[:], trp[:])
                else:
                    nc.scalar.copy(ht[:], trp[:])
                mp = psm.tile([P, fout], F32, tag="mm", name="mp")
                nc.tensor.matmul(mp[:], lhsT=ht[:], rhs=wa_sb[:],
                                 start=True, stop=True)
                if last:
                    nc.vector.tensor_tensor(
                        out=out_strip[:, s * fout:(s + 1) * fout],
                        in0=mp[:], in1=bia_sb[:], op=mybir.AluOpType.add)
                else:
                    nc.scalar.copy(out_strip[:, s * fout:(s + 1) * fout], mp[:])
                    rp = psm.tile([P, fout], F32, tag="mm", name="rp")
                    nc.tensor.matmul(rp[:], lhsT=ht[:], rhs=wb_sb[:],
                                     start=True, stop=True)
                    nc.vector.tensor_tensor(
                        out=rbo_strip[:, s * fout:(s + 1) * fout],
                        in0=rp[:], in1=bia_sb[:], op=mybir.AluOpType.add)

            msg_t = None
            for s in range(TILES_PER_CORE):
                if s % G == 0:
                    gw = min(G, TILES_PER_CORE - s) * TW
                    msg_t = msgp.tile([P, G * TW], F32, tag="msg")
                    nc.sync.dma_start(msg_t[:, :gw],
                                      msg_in.ap()[:, s * TW:s * TW + gw])
                base = (s % G) * TW
                rmode = opts.get("reduce_mode", "reduce_fj")
                if rmode == "tree_jf":
                    # layout [j, fin]: log-tree; widest level on gpsimd
                    h = DEG // 2
                    while h >= 1:
                        eng = nc.gpsimd if (h == DEG // 2 and opts.get("pool_l1", True)) else nc.vector
                        eng.tensor_tensor(
                            out=msg_t[:, base:base + h * fin],
                            in0=msg_t[:, base:base + h * fin],
                            in1=msg_t[:, base + h * fin:base + 2 * h * fin],
                            op=mybir.AluOpType.add)
                        h //= 2
                    s_ap = msg_t[:, base:base + fin]
                elif rmode == "reduce_fj":
                    # layout [fin, j]: one strided reduce over innermost j
                    red = epi.tile([P, fin], F32, tag="red", name="red")
                    nc.vector.tensor_reduce(
                        out=red[:],
                        in_=msg_t[:, base:base + TW].rearrange(
                            "p (f j) -> p f j", f=fin),
                        axis=mybir.AxisListType.X, op=mybir.AluOpType.add)
                    s_ap = red[:]
                else:
                    # layout [fin, j]: gpsimd pre-adds j halves, DVE reduces 16
                    half = TW // 2
                    v3 = msg_t[:, base:base + TW].rearrange(
                        "p (f j) -> p f j", f=fin)
                    nc.gpsimd.tensor_tensor(
                        out=v3[:, :, :DEG // 2], in0=v3[:, :, :DEG // 2],
                        in1=v3[:, :, DEG // 2:], op=mybir.AluOpType.add)
                    red = epi.tile([P, fin], F32, tag="red", name="red")
                    nc.vector.tensor_reduce(
                        out=red[:], in_=v3[:, :, :DEG // 2],
                        axis=mybir.AxisListType.X, op=mybir.AluOpType.add)
                    s_ap = red[:]

                oc = oc_sched[s]
                v = epi.tile([P, fin], F32, tag="v")
                nc.vector.tensor_tensor(
                    out=v[:], in0=s_ap,
                    in1=rb_sb[:, s * fin:(s + 1) * fin],
                    op=mybir.AluOpType.add)
                if oc > 0:
                    f0 = oc_off[s]
                    ot = ovp.tile([P, max(oc_sched) * fin], F32, tag="ovf")
                    (nc.scalar if ovf_act else nc.sync).dma_start(
                        ot[:, :oc * fin],
                        ovm_in.ap()[:, f0 * fin:(f0 + oc) * fin])
                    oh = ohp.tile([P, max(oc_sched) * P], F32, tag="oh")
                    nc.vector.tensor_tensor(
                        out=oh[:, :oc * P].rearrange("p (c d) -> p c d", c=oc),
                        in0=ovd_sb[:, f0:f0 + oc].to_broadcast([P, oc, P]),
                        in1=iota_sb[:, None, :].to_broadcast([P, oc, P]),
                        op=mybir.AluOpType.is_equal)
                    op_ps = pso.tile([P, fin], F32, tag="ops")
                    for c2 in range(oc):
                        nc.tensor.matmul(
                            op_ps[:],
                            lhsT=oh[:, c2 * P:(c2 + 1) * P],
                            rhs=ot[:, c2 * fin:(c2 + 1) * fin],
                            start=(c2 == 0), stop=(c2 == oc - 1))
                    nc.vector.tensor_tensor(out=v[:], in0=v[:], in1=op_ps[:],
                                            op=mybir.AluOpType.add)
                _lrelu_emit(nc, epi, h_sb[:, s * fin:(s + 1) * fin], v[:], fin)
                if merge:
                    emit_tail(s)

            if not merge:
                for s in range(TILES_PER_CORE):
                    emit_tail(s)

            if last:
                nc.sync.dma_start(out_t.ap(), out_strip[:])
            else:
                nc.sync.dma_start(m_out.ap(), out_strip[:])
                nc.sync.dma_start(rb_out.ap(), rbo_strip[:])
    nc.compile()
    return nc


# ---------------------------------------------------------------- runner

class _Results:
    def __init__(self, results):
        self.results = results


class _Runner:
    """Persistent PJRT runner for one compiled Bass program on 8 cores.

    Mirrors bass2jax.run_bass_via_pjrt's multi-core branch but keeps the
    jitted shard_map callable alive across calls (no per-call retrace)."""

    def __init__(self, nc):
        import jax
        from concourse import bass2jax
        from jax.experimental.shard_map import shard_map
        from jax.sharding import Mesh, PartitionSpec

        bass2jax.install_neuronx_cc_hook()
        self.nc = nc
        self.jax = jax
        partition_name = (nc.partition_id_tensor.name
                          if nc.partition_id_tensor else None)
        in_names, out_names, out_avals, zero_info = [], [], [], []
        for alloc in nc.m.functions[0].allocations:
            if not isinstance(alloc, mybir.MemoryLocationSet):
                continue
            name = alloc.memorylocations[0].name
            if alloc.kind == "ExternalInput":
                if name != partition_name:
                    in_names.append(name)
            elif alloc.kind == "ExternalOutput":
                shape = tuple(alloc.tensor_shape)
                dtype = mybir.dt.np(alloc.dtype)
                out_names.append(name)
                out_avals.append(jax.core.ShapedArray(shape, dtype))
                zero_info.append((shape, dtype))
        self.param_names = list(in_names)
        self.out_names = out_names
        self.out_shapes = [z[0] for z in zero_info]
        self.zero_info = zero_info
        n_params, n_outs = len(in_names), len(out_names)
        all_in = in_names + out_names
        if partition_name is not None:
            all_in.append(partition_name)

        def _body(*args):
            operands = list(args)
            if partition_name is not None:
                operands.append(bass2jax.partition_id_tensor())
            outs = bass2jax._bass_exec_p.bind(
                *operands,
                out_avals=tuple(out_avals),
                in_names=tuple(all_in),
                out_names=tuple(out_names),
                lowering_input_output_aliases=(),
                sim_require_finite=True,
                sim_require_nnan=True,
                nc=nc,
            )
            return tuple(outs)

        devices = jax.devices()[:N_CORES]
        self.mesh = Mesh(np.asarray(devices), ("core",))
        self.pspec = PartitionSpec("core")
        in_specs = (self.pspec,) * (n_params + n_outs)
        out_specs = (self.pspec,) * n_outs
        self.fn = jax.jit(
            shard_map(_body, mesh=self.mesh, in_specs=in_specs,
                      out_specs=out_specs, check_rep=False),
            donate_argnums=tuple(range(n_params, n_params + n_outs)),
            keep_unused=True)

    def concat_inputs(self, in_maps):
        return [np.concatenate([np.asarray(in_maps[c][nm])
                                for c in range(N_CORES)], axis=0)
                for nm in self.param_names]

    def zeros(self):
        return [np.zeros((N_CORES * s[0], *s[1:]), d) for s, d in self.zero_info]

    def __call__(self, in_maps):
        outs = self.fn(*self.concat_inputs(in_maps), *self.zeros())
        return _Results([
            {nm: np.asarray(outs[i]).reshape(N_CORES, *self.out_shapes[i])[c]
             for i, nm in enumerate(self.out_names)}
            for c in range(N_CORES)])


def _run(nc, in_maps, tries=3):
    key = id(nc)
    if key not in _cache:
        _cache[key] = _Runner(nc)
    for i in range(tries):
        try:
            return _cache[key](in_maps)
        except Exception:
            if i == tries - 1:
                raise
            time.sleep(10)


def _msgs_from(m_full_ext, SRC_core, fin):
    """Per-core message strips [P, 49*fin*DEG] from the global feature table.

    Device layout per dst tile is [fin, DEG] (feature-major) so a single
    strided tensor_reduce over the innermost DEG axis does the segment sum."""
    out = []
    for c in range(N_CORES):
        g = m_full_ext[SRC_core[c]]          # [P, 49, DEG, fin]
        if OPTS["reduce_mode"] in ("reduce_fj", "split_fj"):
            g = g.transpose(0, 1, 3, 2)      # -> [P, 49, fin, DEG]
        out.append(np.ascontiguousarray(g.reshape(P, -1)))
    return out


def _ovf_msgs_from(m_full_ext, ovf_src, fin):
    out = []
    for c in range(N_CORES):
        g = m_full_ext[ovf_src[c]]           # [P, OC_TOTAL, fin]
        out.append(np.ascontiguousarray(g.reshape(P, -1)))
    return out


def _strip_to_rows(strip, fin):
    """[P, 49*fin] device strip -> [NPC, fin] rows in new-id order."""
    return strip.reshape(P, TILES_PER_CORE, fin).transpose(1, 0, 2).reshape(NPC, fin)


def kernel(x, edge_index, lin1_w, lin1_b, conv1_w_rel, conv1_b_rel,
           conv1_w_root, conv4_w_rel, conv4_b_rel, conv4_w_root,
           pos_w, pos_b):
    if "part" not in _cache:
        _cache["part"] = _build_partition(edge_index)
    part = _cache["part"]

    if "l1" not in _cache:
        _cache["l1"] = _build_l1()
        _cache["l2"] = _build_conv(F2, part["oc_sched"], part["oc_off"],
                                   part["OC_TOTAL"], last=False, opts=OPTS)
        _cache["l3"] = _build_conv(F3, part["oc_sched"], part["oc_off"],
                                   part["OC_TOTAL"], last=True, opts=OPTS)

    new2old = part["new2old"]
    x = np.asarray(x, dtype=np.float32)

    # ---- stage L1 inputs
    w1t = np.ascontiguousarray(np.asarray(lin1_w, np.float32).T)      # [2048,128]
    b1c = np.ascontiguousarray(np.asarray(lin1_b, np.float32).reshape(P, 1))
    w1rel = np.ascontiguousarray(np.asarray(conv1_w_rel, np.float32).T)
    w1root = np.ascontiguousarray(np.asarray(conv1_w_root, np.float32).T)
    b1rel = np.ascontiguousarray(
        np.broadcast_to(np.asarray(conv1_b_rel, np.float32), (P, F2)))

    in_maps1 = []
    for c in range(N_CORES):
        ids = new2old[c * NPC:(c + 1) * NPC]
        xc = np.zeros((NPC, F0), np.float32)
        m = ids >= 0
        xc[m] = x[ids[m]]
        in_maps1.append({
            "xt": np.ascontiguousarray(xc.T),
            "w1t": w1t, "b1": b1c, "w1rel": w1rel, "w1root": w1root,
            "b1rel": b1rel,
        })
    res1 = _run(_cache["l1"], in_maps1)

    m1_full_ext = np.zeros((NPAD + 1, F2), np.float32)
    for c in range(N_CORES):
        m1_full_ext[c * NPC:(c + 1) * NPC] = _strip_to_rows(
            res1.results[c]["m1"], F2)

    # ---- stage L2
    iota = np.ascontiguousarray(
        np.broadcast_to(np.arange(P, dtype=np.float32), (P, P)))
    ident = np.eye(P, dtype=np.float32)
    w4rel = np.ascontiguousarray(np.asarray(conv4_w_rel, np.float32).T)
    w4root = np.ascontiguousarray(np.asarray(conv4_w_root, np.float32).T)
    b4rel = np.ascontiguousarray(
        np.broadcast_to(np.asarray(conv4_b_rel, np.float32), (P, F3)))

    msgs1 = _msgs_from(m1_full_ext, part["SRC_core"], F2)
    ovfs1 = _ovf_msgs_from(m1_full_ext, part["ovf_src"], F2)
    in_maps2 = []
    for c in range(N_CORES):
        in_maps2.append({
            "msg": msgs1[c],
            "rb": res1.results[c]["r1b"],
            "ovm": ovfs1[c],
            "ovd": np.ascontiguousarray(part["ovf_dstrel"][c]),
            "iota": iota, "ident": ident,
            "wa": w4rel, "wb": w4root, "bia": b4rel,
        })
    res2 = _run(_cache["l2"], in_maps2)

    m2_full_ext = np.zeros((NPAD + 1, F3), np.float32)
    for c in range(N_CORES):
        m2_full_ext[c * NPC:(c + 1) * NPC] = _strip_to_rows(
            res2.results[c]["m"], F3)

    # ---- stage L3
    posw = np.ascontiguousarray(np.asarray(pos_w, np.float32).T)      # [32, 3]
    posb = np.ascontiguousarray(
        np.broadcast_to(np.asarray(pos_b, np.float32), (P, 3)))
    msgs2 = _msgs_from(m2_full_ext, part["SRC_core"], F3)
    ovfs2 = _ovf_msgs_from(m2_full_ext, part["ovf_src"], F3)
    in_maps3 = []
    for c in range(N_CORES):
        in_maps3.append({
            "msg": msgs2[c],
            "rb": res2.results[c]["rbo"],
            "ovm": ovfs2[c],
            "ovd": np.ascontiguousarray(part["ovf_dstrel"][c]),
            "iota": iota, "ident": ident,
            "wa": posw, "bia": posb,
        })
    res3 = _run(_cache["l3"], in_maps3)
    _cache["last_inmaps"] = {"l1": in_maps1, "l2": in_maps2, "l3": in_maps3}

    pos_full = np.empty((NPAD, 3), np.float32)
    for c in range(N_CORES):
        pos_full[c * NPC:(c + 1) * NPC] = _strip_to_rows(
            res3.results[c]["pos"], 3)

    out = np.empty((N_NODES, 3), np.float32)
    valid = new2old >= 0
    out[new2old[valid]] = pos_full[valid]
    return out
